# revision 1
# baseline (speedup 1.0000x reference)
"""Trainium2 Bass kernel for a 12-layer dense transformer encoder
(B=16, T=512, C=1024, H=16, F=4096, V=30522), data-parallel over batch
across 8 NeuronCores (2 sequences per core).

kernel(**inputs) takes the FULL unsharded inputs (as produced by the
reference setup_inputs()) and returns the FULL [16, 512] float32 output.

Host side: embedding gather + positional add, LayerNorm weight folding into
the adjacent matmul weights (exact for the reference parametrization),
weight pre-tiling and bf16 cast.  Device side (identical SPMD program on all
8 cores): the full transformer stack on SBUF-resident activations with bf16
TensorEngine matmuls and fp32 accumulation/residual stream.
"""
import numpy as np
from contextlib import ExitStack

import ml_dtypes

import concourse.bass as bass
import concourse.mybir as mybir
import concourse.tile as tile
from concourse import bacc

F32 = mybir.dt.float32
BF16 = mybir.dt.bfloat16
AF = mybir.ActivationFunctionType
ALU = mybir.AluOpType

TOK, C, H, D, FF = 1024, 1024, 16, 64, 4096
NT = TOK // 128
NCt = C // 128
NFt = FF // 128
EPS = 1e-5
NCORES = 8


def build_program(layers=12, repeat=1):
    nc = bacc.Bacc("TRN2", target_bir_lowering=False, debug=False)
    LL = layers

    x0 = nc.dram_tensor("x0", (TOK, C), F32, kind="ExternalInput")
    wq = nc.dram_tensor("wq", (LL, NCt, NCt, 128, 128), BF16, kind="ExternalInput")
    wk = nc.dram_tensor("wk", (LL, NCt, NCt, 128, 128), BF16, kind="ExternalInput")
    wv = nc.dram_tensor("wv", (LL, NCt, 2, 128, 512), BF16, kind="ExternalInput")
    wo = nc.dram_tensor("wo", (LL, NCt, 2, 128, 512), BF16, kind="ExternalInput")
    w1 = nc.dram_tensor("w1", (LL, NCt, NFt, 128, 128), BF16, kind="ExternalInput")
    w2 = nc.dram_tensor("w2", (LL, NFt, 2, 128, 512), BF16, kind="ExternalInput")
    bq = nc.dram_tensor("bq", (LL, NCt, 128), F32, kind="ExternalInput")
    bk = nc.dram_tensor("bk", (LL, NCt, 128), F32, kind="ExternalInput")
    bv = nc.dram_tensor("bv", (LL, C), F32, kind="ExternalInput")
    bo = nc.dram_tensor("bo", (LL, C), F32, kind="ExternalInput")
    b1 = nc.dram_tensor("b1", (LL, NFt, 128), F32, kind="ExternalInput")
    b2 = nc.dram_tensor("b2", (LL, C), F32, kind="ExternalInput")
    hw = nc.dram_tensor("hw", (NCt, 128, 1), BF16, kind="ExternalInput")
    hb = nc.dram_tensor("hb", (1, 1), F32, kind="ExternalInput")
    yo = nc.dram_tensor("y", (1, TOK), F32, kind="ExternalOutput")

    with tile.TileContext(nc) as tc, ExitStack() as ctx:
        px = ctx.enter_context(tc.tile_pool(name="px", bufs=1))
        pchunk = ctx.enter_context(tc.tile_pool(name="pchunk", bufs=6))
        pmov = ctx.enter_context(tc.tile_pool(name="pmov", bufs=4))
        pbias = ctx.enter_context(tc.tile_pool(name="pbias", bufs=2))
        pmisc = ctx.enter_context(tc.tile_pool(name="pmisc", bufs=4))
        pr = ctx.enter_context(tc.tile_pool(name="pr", bufs=2))
        ppsum = ctx.enter_context(tc.tile_pool(name="ppsum", bufs=8, space="PSUM"))

        xs, hs, hTs, vaug, big = [], [], [], [], []
        for t in range(NT):
            xs.append(px.tile([128, C], F32, tag=f"x{t}", name=f"x{t}"))
        for t in range(NT):
            hs.append(px.tile([128, C], BF16, tag=f"h{t}", name=f"h{t}"))
            hTs.append(px.tile([128, TOK], BF16, tag=f"hT{t}", name=f"hT{t}"))
        for t in range(NT):
            vaug.append(px.tile([128, H, D + 1], BF16, tag=f"v{t}", name=f"v{t}"))
        for t in range(NFt):
            big.append(px.tile([128, TOK], BF16, tag=f"big{t}", name=f"big{t}"))
        qT, kT, yT = big[0:8], big[8:16], big[16:24]
        sslot = []
        for i in range(8):
            sslot.append(big[24 + i][:, 0:512])
            sslot.append(big[24 + i][:, 512:1024])

        eps_t = pmisc.tile([128, 1], F32, tag="eps", name="eps_t")
        nc.vector.memset(eps_t[:], EPS)

        def ln_into(dst_tiles):
            for t in range(NT):
                xr = xs[t][:].rearrange("p (s f) -> p s f", s=2)
                stats = pmisc.tile([128, 2, 6], F32, tag="stats", name="stats")
                for sgi in range(2):
                    nc.vector.bn_stats(stats[:, sgi, :], xr[:, sgi, :])
                mv = pmisc.tile([128, 2], F32, tag="mv", name="mv")
                nc.vector.bn_aggr(mv[:], stats[:])
                sd = pmisc.tile([128, 1], F32, tag="sd", name="sd")
                nc.scalar.activation(sd[:], mv[:, 1:2], AF.Sqrt, bias=eps_t[:])
                rstd = pmisc.tile([128, 1], F32, tag="rstd", name="rstd")
                nc.vector.reciprocal(rstd[:], sd[:])
                nc.vector.tensor_scalar(dst_tiles[t][:], xs[t][:],
                                        mv[:, 0:1], rstd[:],
                                        ALU.subtract, ALU.mult)

        def transpose_into(hT_tiles, src_tiles):
            for ct in range(NCt):
                for t in range(NT):
                    nc.sync.dma_start_transpose(
                        hT_tiles[ct][:, t * 128:(t + 1) * 128],
                        src_tiles[t][:, ct * 128:(ct + 1) * 128])

        def layer(l):
            ln_into(hs)
            transpose_into(hTs, hs)

            # q,k projections: W-stationary -> qT,kT [C_out, tok]
            bq_sb = pbias.tile([128, NCt], F32, tag="bq", name="bq_sb")
            bk_sb = pbias.tile([128, NCt], F32, tag="bk", name="bk_sb")
            nc.sync.dma_start(bq_sb[:], bq.ap()[l].rearrange("m p -> p m"))
            nc.sync.dma_start(bk_sb[:], bk.ap()[l].rearrange("m p -> p m"))
            for m in range(NCt):
                pq0 = ppsum.tile([128, 512], F32, tag="p", name="pq0")
                pq1 = ppsum.tile([128, 512], F32, tag="p", name="pq1")
                pk0 = ppsum.tile([128, 512], F32, tag="p", name="pk0")
                pk1 = ppsum.tile([128, 512], F32, tag="p", name="pk1")
                for k in range(NCt):
                    cq = pchunk.tile([128, 128], BF16, tag="cq", name="cq")
                    ck = pchunk.tile([128, 128], BF16, tag="ck", name="ck")
                    nc.sync.dma_start(cq[:], wq.ap()[l, k, m])
                    nc.sync.dma_start(ck[:], wk.ap()[l, k, m])
                    st, sp = (k == 0), (k == NCt - 1)
                    nc.tensor.matmul(pq0[:], cq[:], hTs[k][:, 0:512], start=st, stop=sp)
                    nc.tensor.matmul(pq1[:], cq[:], hTs[k][:, 512:1024], start=st, stop=sp)
                    nc.tensor.matmul(pk0[:], ck[:], hTs[k][:, 0:512], start=st, stop=sp)
                    nc.tensor.matmul(pk1[:], ck[:], hTs[k][:, 512:1024], start=st, stop=sp)
                nc.vector.tensor_scalar(qT[m][:, 0:512], pq0[:], bq_sb[:, m:m + 1],
                                        None, ALU.add)
                nc.vector.tensor_scalar(qT[m][:, 512:1024], pq1[:], bq_sb[:, m:m + 1],
                                        None, ALU.add)
                nc.vector.tensor_scalar(kT[m][:, 0:512], pk0[:], bk_sb[:, m:m + 1],
                                        None, ALU.add)
                nc.vector.tensor_scalar(kT[m][:, 512:1024], pk1[:], bk_sb[:, m:m + 1],
                                        None, ALU.add)

            # v projection: hT-stationary -> natural v, augmented with ones col
            for blk in range(2):
                bv_rep = pbias.tile([128, 512], F32, tag="bv", name="bv_rep")
                nc.sync.dma_start(
                    bv_rep[:],
                    bv.ap()[l:l + 1, blk * 512:(blk + 1) * 512].to_broadcast((128, 512)))
                pvs = []
                for t in range(NT):
                    pv = ppsum.tile([128, 512], F32, tag="p", name="pv")
                    pvs.append(pv)
                for k in range(NCt):
                    cv = pmov.tile([128, 512], BF16, tag="cv", name="cv")
                    nc.sync.dma_start(cv[:], wv.ap()[l, k, blk])
                    for t in range(NT):
                        nc.tensor.matmul(pvs[t][:], hTs[k][:, t * 128:(t + 1) * 128],
                                         cv[:], start=(k == 0), stop=(k == NCt - 1))
                for t in range(NT):
                    nc.vector.tensor_tensor(
                        vaug[t][:, blk * 8:(blk + 1) * 8, 0:D],
                        pvs[t][:].rearrange("p (h d) -> p h d", h=8),
                        bv_rep[:].rearrange("p (h d) -> p h d", h=8),
                        ALU.add)
            for t in range(NT):
                nc.vector.memset(vaug[t][:, :, D:D + 1], 1.0)

            # attention, per (sequence b, head h)
            for b in range(2):
                for h in range(H):
                    ct, r0 = h // 2, 64 * (h % 2)
                    kTh = kT[ct][r0:r0 + 64, b * 512:(b + 1) * 512]
                    qTh = qT[ct][r0:r0 + 64, b * 512:(b + 1) * 512]
                    stiles = []
                    for tkt in range(4):
                        ps = ppsum.tile([128, 512], F32, tag="p", name="ps")
                        nc.tensor.matmul(ps[:],
                                         kTh[:, tkt * 128:(tkt + 1) * 128],
                                         qTh, start=True, stop=True)
                        ssb = sslot[((b * H + h) % 4) * 4 + tkt]
                        nc.scalar.activation(ssb, ps[:], AF.Exp)
                        stiles.append(ssb)
                    py = ppsum.tile([65, 512], F32, tag="p", name="py")
                    for tkt in range(4):
                        nc.tensor.matmul(py[:], vaug[b * 4 + tkt][:, h, :],
                                         stiles[tkt], start=(tkt == 0),
                                         stop=(tkt == 3))
                    rz = pr.tile([1, 512], F32, tag="rz", name="rz")
                    nc.vector.reciprocal(rz[:], py[64:65, :])
                    rz_rep = pr.tile([64, 512], F32, tag="rzr", name="rz_rep")
                    nc.gpsimd.partition_broadcast(rz_rep[:], rz[:])
                    nc.vector.tensor_tensor(
                        yT[ct][r0:r0 + 64, b * 512:(b + 1) * 512],
                        py[0:64, :], rz_rep[:], ALU.mult)

            # output projection: yT-stationary -> natural, residual add
            for blk in range(2):
                bo_rep = pbias.tile([128, 512], F32, tag="bo", name="bo_rep")
                nc.sync.dma_start(
                    bo_rep[:],
                    bo.ap()[l:l + 1, blk * 512:(blk + 1) * 512].to_broadcast((128, 512)))
                pps = []
                for t in range(NT):
                    pp = ppsum.tile([128, 512], F32, tag="p", name="pp")
                    pps.append(pp)
                for k in range(NCt):
                    co = pmov.tile([128, 512], BF16, tag="co", name="co")
                    nc.sync.dma_start(co[:], wo.ap()[l, k, blk])
                    for t in range(NT):
                        nc.tensor.matmul(pps[t][:], yT[k][:, t * 128:(t + 1) * 128],
                                         co[:], start=(k == 0), stop=(k == NCt - 1))
                for t in range(NT):
                    xsl = xs[t][:, blk * 512:(blk + 1) * 512]
                    nc.vector.tensor_tensor(xsl, xsl, pps[t][:], ALU.add)
                    nc.vector.tensor_tensor(xsl, xsl, bo_rep[:], ALU.add)

            ln_into(hs)
            transpose_into(hTs, hs)

            # W1 + bias + gelu: W-stationary -> gT
            b1_sb = pbias.tile([128, NFt], F32, tag="b1", name="b1_sb")
            nc.sync.dma_start(b1_sb[:], b1.ap()[l].rearrange("m p -> p m"))
            for ft in range(NFt):
                pg0 = ppsum.tile([128, 512], F32, tag="p", name="pg0")
                pg1 = ppsum.tile([128, 512], F32, tag="p", name="pg1")
                for k in range(NCt):
                    c1 = pchunk.tile([128, 128], BF16, tag="c1", name="c1")
                    nc.sync.dma_start(c1[:], w1.ap()[l, k, ft])
                    st, sp = (k == 0), (k == NCt - 1)
                    nc.tensor.matmul(pg0[:], c1[:], hTs[k][:, 0:512], start=st, stop=sp)
                    nc.tensor.matmul(pg1[:], c1[:], hTs[k][:, 512:1024], start=st, stop=sp)
                for tb, pg in ((0, pg0), (1, pg1)):
                    dst = big[ft][:, tb * 512:(tb + 1) * 512]
                    nc.scalar.activation(dst, pg[:], AF.Gelu,
                                         bias=b1_sb[:, ft:ft + 1])

            # W2: gT-stationary -> natural, residual add
            for blk in range(2):
                b2_rep = pbias.tile([128, 512], F32, tag="b2", name="b2_rep")
                nc.sync.dma_start(
                    b2_rep[:],
                    b2.ap()[l:l + 1, blk * 512:(blk + 1) * 512].to_broadcast((128, 512)))
                pws = []
                for t in range(NT):
                    pw = ppsum.tile([128, 512], F32, tag="p", name="pw")
                    pws.append(pw)
                for k in range(NFt):
                    c2 = pmov.tile([128, 512], BF16, tag="c2", name="c2")
                    nc.sync.dma_start(c2[:], w2.ap()[l, k, blk])
                    for t in range(NT):
                        nc.tensor.matmul(pws[t][:], big[k][:, t * 128:(t + 1) * 128],
                                         c2[:], start=(k == 0), stop=(k == NFt - 1))
                for t in range(NT):
                    xsl = xs[t][:, blk * 512:(blk + 1) * 512]
                    nc.vector.tensor_tensor(xsl, xsl, pws[t][:], ALU.add)
                    nc.vector.tensor_tensor(xsl, xsl, b2_rep[:], ALU.add)

        def whole_net(_iv=None):
            for t in range(NT):
                nc.sync.dma_start(xs[t][:], x0.ap()[t * 128:(t + 1) * 128, :])
            for l in range(layers):
                layer(l)
            ln_into(hs)
            transpose_into(hTs, hs)
            hw_sb = pbias.tile([128, NCt], BF16, tag="hw", name="hw_sb")
            nc.sync.dma_start(hw_sb[:], hw.ap().rearrange("m p one -> p (m one)"))
            hb_sb = pmisc.tile([1, 1], F32, tag="hb", name="hb_sb")
            nc.sync.dma_start(hb_sb[:], hb.ap())
            y_sb = pr.tile([1, TOK], F32, tag="ysb", name="y_sb")
            for tb in range(2):
                ph = ppsum.tile([1, 512], F32, tag="p", name="ph")
                for k in range(NCt):
                    nc.tensor.matmul(ph[:], hw_sb[:, k:k + 1],
                                     hTs[k][:, tb * 512:(tb + 1) * 512],
                                     start=(k == 0), stop=(k == NCt - 1))
                # softplus(z) = ln(1 + exp(z)); z is bounded (~|z|<6) here
                ez = pr.tile([1, 512], F32, tag="ez", name="ez")
                nc.scalar.activation(ez[:], ph[:], AF.Exp, bias=hb_sb[:])
                nc.vector.tensor_scalar(ez[:], ez[:], 1.0, None, ALU.add)
                nc.scalar.activation(y_sb[0:1, tb * 512:(tb + 1) * 512], ez[:],
                                     AF.Ln)
            nc.sync.dma_start(yo.ap(), y_sb[:])

        if repeat == 1:
            whole_net()
        else:
            tc.For_i_unrolled(0, repeat, 1, whole_net, max_unroll=1)

    nc.compile()
    return nc


def prep_weights(inputs, layers=12):
    bf16 = ml_dtypes.bfloat16
    f32 = np.float32

    ln1_w, ln1_b = np.asarray(inputs["ln1_w"], f32), np.asarray(inputs["ln1_b"], f32)
    ln2_w, ln2_b = np.asarray(inputs["ln2_w"], f32), np.asarray(inputs["ln2_b"], f32)
    out = {}

    def tile_stat(w):  # [C_in, M] -> [C_in/128, M/128, 128, 128]
        ci, m = w.shape
        return np.ascontiguousarray(
            w.reshape(ci // 128, 128, m // 128, 128).transpose(0, 2, 1, 3))

    def tile_mov(w):  # [K, N] -> [K/128, N/512, 128, 512]
        k, n = w.shape
        return np.ascontiguousarray(
            w.reshape(k // 128, 128, n // 512, 512).transpose(0, 2, 1, 3))

    L = layers
    scale = f32(1.0) / np.sqrt(f32(D))
    wq_t = np.empty((L, NCt, NCt, 128, 128), bf16)
    wk_t = np.empty((L, NCt, NCt, 128, 128), bf16)
    wv_t = np.empty((L, NCt, 2, 128, 512), bf16)
    wo_t = np.empty((L, NCt, 2, 128, 512), bf16)
    w1_t = np.empty((L, NCt, NFt, 128, 128), bf16)
    w2_t = np.empty((L, NFt, 2, 128, 512), bf16)
    bq_t = np.empty((L, NCt, 128), f32)
    bk_t = np.empty((L, NCt, 128), f32)
    bv_t = np.empty((L, C), f32)
    bo_t = np.empty((L, C), f32)
    b1_t = np.empty((L, NFt, 128), f32)
    b2_t = np.empty((L, C), f32)
    for l in range(L):
        Wq = np.asarray(inputs["Wq"][l], f32)
        Wk = np.asarray(inputs["Wk"][l], f32)
        Wv = np.asarray(inputs["Wv"][l], f32)
        W1 = np.asarray(inputs["W1"][l], f32)
        d1 = ln1_w[l][:, None]
        wq_t[l] = tile_stat((d1 * Wq) * scale)
        wk_t[l] = tile_stat(d1 * Wk)
        wv_t[l] = tile_mov(d1 * Wv)
        wo_t[l] = tile_mov(np.asarray(inputs["Wo"][l], f32))
        w1_t[l] = tile_stat(ln2_w[l][:, None] * W1)
        w2_t[l] = tile_mov(np.asarray(inputs["W2"][l], f32))
        bq_t[l] = ((ln1_b[l] @ Wq + np.asarray(inputs["bq"][l], f32))
                   * scale).reshape(NCt, 128)
        bk_t[l] = (ln1_b[l] @ Wk + np.asarray(inputs["bk"][l], f32)).reshape(NCt, 128)
        bv_t[l] = ln1_b[l] @ Wv + np.asarray(inputs["bv"][l], f32)
        bo_t[l] = np.asarray(inputs["bo"][l], f32)
        b1_t[l] = (ln2_b[l] @ W1 + np.asarray(inputs["b1"][l], f32)).reshape(NFt, 128)
        b2_t[l] = np.asarray(inputs["b2"][l], f32)

    head_w = np.asarray(inputs["head_w"], f32)
    hw_f = np.asarray(inputs["ln_f_w"], f32)[:, None] * head_w
    hb_f = (np.asarray(inputs["ln_f_b"], f32) @ head_w
            + np.asarray(inputs["head_b"], f32))

    out["wq"], out["wk"], out["wv"], out["wo"] = wq_t, wk_t, wv_t, wo_t
    out["w1"], out["w2"] = w1_t, w2_t
    out["bq"], out["bk"], out["bv"], out["bo"] = bq_t, bk_t, bv_t, bo_t
    out["b1"], out["b2"] = b1_t, b2_t
    out["hw"] = np.ascontiguousarray(hw_f.reshape(NCt, 128, 1)).astype(bf16)
    out["hb"] = hb_f.reshape(1, 1)
    return out


def prep_x0(inputs):
    idx = np.asarray(inputs["idx"])
    tok = np.asarray(inputs["tok_emb"], np.float32)
    pos = np.asarray(inputs["pos_emb"], np.float32)
    x0 = tok[idx] + pos  # [B, T, C]
    return [np.ascontiguousarray(x0[2 * c:2 * c + 2].reshape(TOK, C))
            for c in range(NCORES)]


class SpmdRunner:
    """Executes a compiled Bass module on the 8 axon-attached NeuronCores via
    PJRT (modeled on concourse.bass2jax.run_bass_via_pjrt, but jits once and
    keeps inputs device-resident so repeated calls are cheap)."""

    def __init__(self, nc, n_cores=NCORES):
        import jax
        from jax.sharding import Mesh, PartitionSpec
        from jax.experimental.shard_map import shard_map
        from concourse import bass2jax
        from concourse.bass2jax import _bass_exec_p, install_neuronx_cc_hook

        install_neuronx_cc_hook()
        self.jax = jax
        self.nc = nc
        self.n_cores = n_cores
        self.PartitionSpec = PartitionSpec

        partition_name = (
            nc.partition_id_tensor.name if nc.partition_id_tensor else None)
        in_names, out_names, out_avals = [], [], []
        self.extra_zero_names = []
        for alloc in nc.m.functions[0].allocations:
            if not isinstance(alloc, mybir.MemoryLocationSet):
                continue
            name = alloc.memorylocations[0].name
            if alloc.kind == "ExternalInput":
                if name != partition_name:
                    in_names.append(name)
            elif alloc.kind == "ExternalOutput":
                out_names.append(name)
                out_avals.append(jax.core.ShapedArray(
                    tuple(alloc.tensor_shape), mybir.dt.np(alloc.dtype)))
        if nc.dbg_addr is not None:
            self.extra_zero_names.append(nc.dbg_addr.name)

        self.in_names = list(in_names)
        self.out_names = out_names
        self.out_avals = out_avals
        n_params = len(in_names) + len(self.extra_zero_names)
        n_outs = len(out_avals)
        all_in_names = list(in_names) + self.extra_zero_names + list(out_names)
        if partition_name is not None:
            all_in_names.append(partition_name)

        def _body(*args):
            operands = list(args)
            if partition_name is not None:
                operands.append(bass2jax.partition_id_tensor())
            outs = _bass_exec_p.bind(
                *operands,
                out_avals=tuple(out_avals),
                in_names=tuple(all_in_names),
                out_names=tuple(out_names),
                lowering_input_output_aliases=(),
                sim_require_finite=True,
                sim_require_nnan=True,
                nc=nc,
            )
            return tuple(outs)

        devices = jax.devices()[:n_cores]
        assert len(devices) >= n_cores or len(devices) == n_cores, (
            f"need {n_cores} neuron cores, found {len(devices)}")
        self.mesh = Mesh(np.asarray(devices), ("core",))
        in_specs = (PartitionSpec("core"),) * (n_params + n_outs)
        out_specs = (PartitionSpec("core"),) * n_outs
        self.fn = jax.jit(
            shard_map(_body, mesh=self.mesh, in_specs=in_specs,
                      out_specs=out_specs, check_rep=False),
            keep_unused=True)
        self._dev_args = None

    def place_inputs(self, in_maps):
        jax = self.jax
        sharding = jax.sharding.NamedSharding(
            self.mesh, self.PartitionSpec("core"))
        args = []
        for name in self.in_names:
            concat = np.concatenate(
                [np.asarray(in_maps[c][name]) for c in range(self.n_cores)],
                axis=0)
            args.append(jax.device_put(concat, sharding))
        for name in self.extra_zero_names:
            args.append(jax.device_put(
                np.zeros((self.n_cores, 2), np.uint32), sharding))
        for aval in self.out_avals:
            args.append(jax.device_put(
                np.zeros((self.n_cores * aval.shape[0], *aval.shape[1:]),
                         aval.dtype), sharding))
        self._dev_args = args

    def run(self):
        outs = self.fn(*self._dev_args)
        self.jax.block_until_ready(outs)
        return outs

    def results(self, outs):
        per_core = []
        for c in range(self.n_cores):
            d = {}
            for i, name in enumerate(self.out_names):
                aval = self.out_avals[i]
                d[name] = np.asarray(outs[i]).reshape(
                    self.n_cores, *aval.shape)[c]
            per_core.append(d)
        return per_core


_CACHE = {}


def _get_runner(repeat=1):
    key = ("prog", repeat)
    if key not in _CACHE:
        ncb = build_program(layers=12, repeat=repeat)
        _CACHE[key] = SpmdRunner(ncb, NCORES)
    return _CACHE[key]


def kernel(**inputs) -> np.ndarray:
    w = prep_weights(inputs, layers=12)
    x0s = prep_x0(inputs)
    runner = _get_runner(repeat=1)
    in_maps = [dict(w, x0=x0s[c]) for c in range(NCORES)]
    runner.place_inputs(in_maps)
    outs = runner.run()
    res = runner.results(outs)
    y = np.stack([res[c]["y"].reshape(2, 512) for c in range(NCORES)])
    return np.ascontiguousarray(y.reshape(16, 512).astype(np.float32))



# revision 34
# speedup vs baseline: 1.0168x; 1.0168x over previous
"""Trainium2 Bass kernel for a 12-layer dense transformer encoder
(B=16, T=512, C=1024, H=16, F=4096, V=30522), data-parallel over batch
across 8 NeuronCores (2 sequences per core).

kernel(**inputs) takes the FULL unsharded inputs (as produced by the
reference setup_inputs()) and returns the FULL [16, 512] float32 output.

Host side: embedding gather + positional add + transpose, LayerNorm weight
folding into the adjacent matmul weights (exact for any parametrization),
weight pre-tiling and bf16 cast.

Device side (identical SPMD program on all 8 cores): the residual stream is
kept TRANSPOSED (xT [C, tok]) on SBUF for the whole network, so no on-chip
transposes are ever needed:
  - LayerNorm stats (per-token mean/var over C) via ones-stationary matmuls
    on the TensorEngine, normalization applied with partition-broadcast
    mean/rstd rows; rstd computed as exp(-0.5*ln(var+eps)) so the ACT table
    set (natural_log_exp) is shared with the attention exp and the final
    softplus -- no table-set thrash.
  - Q/K/O/W1/W2 are weight-stationary matmuls producing transposed outputs
    directly; biases become per-partition scalars fused into the PSUM
    evacuation op.  V is hT-stationary producing natural v for the AV
    matmuls (ones-row augmented for the softmax denominator).
  - Attention exp on 2-bank [128,1024] PSUM pairs (one ACT op for both
    sequences), softmax normalization via reciprocal_approx_fast.
"""
import numpy as np
from contextlib import ExitStack

import ml_dtypes

import concourse.bass as bass
import concourse.mybir as mybir
import concourse.tile as tile
from concourse import bacc

F32 = mybir.dt.float32
BF16 = mybir.dt.bfloat16
AF = mybir.ActivationFunctionType
ALU = mybir.AluOpType

TOK, C, H, D, FF = 1024, 1024, 16, 64, 4096
NT = TOK // 128    # tok tiles
NCt = C // 128     # channel tiles
NFt = FF // 128    # ff tiles
EPS = 1e-5
NCORES = 8
L = 12


def build_program(layers=L, repeat=1, dbg=False):
    nc = bacc.Bacc("TRN2", target_bir_lowering=False, debug=False)
    LL = layers
    dbg_t = {}
    if dbg:
        dbg_t["hs1"] = nc.dram_tensor("dbg_hs1", (C, TOK), BF16, kind="ExternalOutput")
        dbg_t["qT"] = nc.dram_tensor("dbg_qT", (C, TOK), BF16, kind="ExternalOutput")
        dbg_t["kT"] = nc.dram_tensor("dbg_kT", (C, TOK), BF16, kind="ExternalOutput")
        dbg_t["va"] = nc.dram_tensor("dbg_va", (TOK, H * (D + 1)), BF16, kind="ExternalOutput")
        dbg_t["yT"] = nc.dram_tensor("dbg_yT", (C, TOK), BF16, kind="ExternalOutput")
        dbg_t["xo"] = nc.dram_tensor("dbg_xo", (C, TOK), F32, kind="ExternalOutput")
        dbg_t["gT"] = nc.dram_tensor("dbg_gT", (FF, TOK), BF16, kind="ExternalOutput")
        dbg_t["xl"] = nc.dram_tensor("dbg_xl", (C, TOK), F32, kind="ExternalOutput")
        dbg_t["stt"] = nc.dram_tensor("dbg_stt", (128, TOK), F32, kind="ExternalOutput")
        dbg_t["mb"] = nc.dram_tensor("dbg_mb", (128, TOK), BF16, kind="ExternalOutput")
        dbg_t["rb"] = nc.dram_tensor("dbg_rb", (128, TOK), BF16, kind="ExternalOutput")
        dbg_t["s00"] = nc.dram_tensor("dbg_s00", (128, TOK), BF16, kind="ExternalOutput")
        dbg_t["s10"] = nc.dram_tensor("dbg_s10", (128, TOK), BF16, kind="ExternalOutput")
        dbg_t["py0"] = nc.dram_tensor("dbg_py0", (65, TOK), F32, kind="ExternalOutput")
        dbg_t["rz0"] = nc.dram_tensor("dbg_rz0", (1, TOK), F32, kind="ExternalOutput")
        dbg_t["rzr0"] = nc.dram_tensor("dbg_rzr0", (64, TOK), BF16, kind="ExternalOutput")

    x0 = nc.dram_tensor("x0", (C, TOK), F32, kind="ExternalInput")
    wq = nc.dram_tensor("wq", (LL, NCt, 128, NCt, 128), BF16, kind="ExternalInput")
    wk = nc.dram_tensor("wk", (LL, NCt, 128, NCt, 128), BF16, kind="ExternalInput")
    wv = nc.dram_tensor("wv", (LL, NCt, 128, C), BF16, kind="ExternalInput")
    wo = nc.dram_tensor("wo", (LL, NCt, 128, NCt, 128), BF16, kind="ExternalInput")
    w1 = nc.dram_tensor("w1", (LL, NFt, 128, NCt, 128), BF16, kind="ExternalInput")
    w2 = nc.dram_tensor("w2", (LL, NCt, 128, NFt, 128), BF16, kind="ExternalInput")
    bq = nc.dram_tensor("bq", (128, LL * NCt), F32, kind="ExternalInput")
    bk = nc.dram_tensor("bk", (128, LL * NCt), F32, kind="ExternalInput")
    bv = nc.dram_tensor("bv", (LL, C), BF16, kind="ExternalInput")
    bo = nc.dram_tensor("bo", (128, LL * NCt), F32, kind="ExternalInput")
    b1 = nc.dram_tensor("b1", (128, LL * NFt), F32, kind="ExternalInput")
    b2 = nc.dram_tensor("b2", (128, LL * NCt), F32, kind="ExternalInput")
    hw = nc.dram_tensor("hw", (128, NCt), BF16, kind="ExternalInput")
    hb = nc.dram_tensor("hb", (1, 1), F32, kind="ExternalInput")
    yo = nc.dram_tensor("y", (1, TOK), F32, kind="ExternalOutput")

    with tile.TileContext(nc) as tc, ExitStack() as ctx:
        px = ctx.enter_context(tc.tile_pool(name="px", bufs=1))
        pwqk = ctx.enter_context(tc.tile_pool(name="pwqk", bufs=2))
        pwv = ctx.enter_context(tc.tile_pool(name="pwv", bufs=1))
        pwo = ctx.enter_context(tc.tile_pool(name="pwo", bufs=2))
        pw1 = ctx.enter_context(tc.tile_pool(name="pw1", bufs=2))
        pw2 = ctx.enter_context(tc.tile_pool(name="pw2", bufs=2))
        pbias = ctx.enter_context(tc.tile_pool(name="pbias", bufs=1))
        pmisc = ctx.enter_context(tc.tile_pool(name="pmisc", bufs=2))
        pr = ctx.enter_context(tc.tile_pool(name="pr", bufs=1))
        pb16 = ctx.enter_context(tc.tile_pool(name="pb16", bufs=1))
        pp1 = ctx.enter_context(tc.tile_pool(name="pp1", bufs=4, space="PSUM"))
        pp2 = ctx.enter_context(tc.tile_pool(name="pp2", bufs=2, space="PSUM"))

        # persistent SBUF tiles
        xs, hs, vaug, big = [], [], [], []
        for t in range(NCt):
            xs.append(px.tile([128, TOK], F32, tag=f"x{t}", name=f"x{t}"))
        for t in range(NCt):
            hs.append(px.tile([128, TOK], BF16, tag=f"h{t}", name=f"h{t}"))
        for t in range(NT):
            vaug.append(px.tile([128, H, D + 1], BF16, tag=f"v{t}", name=f"v{t}"))
        for i in range(32):
            big.append(px.tile([128, TOK], BF16, tag=f"big{i}", name=f"big{i}"))
        qT, kT, yT, sslot = big[0:8], big[8:16], big[16:24], big[24:32]

        ones_sb = pbias.tile([128, 1], BF16, tag="ones", name="ones_sb")
        nc.vector.memset(ones_sb[:], 1.0)
        eps_t = pbias.tile([1, 1], F32, tag="eps", name="eps_t")
        nc.vector.memset(eps_t[:], EPS)
        for t in range(NT):
            nc.vector.memset(vaug[t][:, :, D:D + 1], 1.0)

        # preload all biases
        bq_sb = pbias.tile([128, LL * NCt], F32, tag="bq", name="bq_sb")
        bk_sb = pbias.tile([128, LL * NCt], F32, tag="bk", name="bk_sb")
        bo_sb = pbias.tile([128, LL * NCt], F32, tag="bo", name="bo_sb")
        b2_sb = pbias.tile([128, LL * NCt], F32, tag="b2", name="b2_sb")
        b1_sb = pbias.tile([128, LL * NFt], F32, tag="b1", name="b1_sb")
        hw_sb = pbias.tile([128, NCt], BF16, tag="hw", name="hw_sb")
        hb_sb = pbias.tile([1, 1], F32, tag="hb", name="hb_sb")
        nc.sync.dma_start(bq_sb[:], bq.ap())
        nc.sync.dma_start(bk_sb[:], bk.ap())
        nc.sync.dma_start(bo_sb[:], bo.ap())
        nc.sync.dma_start(b2_sb[:], b2.ap())
        nc.sync.dma_start(b1_sb[:], b1.ap())
        nc.sync.dma_start(hw_sb[:], hw.ap())
        nc.sync.dma_start(hb_sb[:], hb.ap())

        def ln_transposed():
            """x (xs tiles, [C,tok] f32) -> normalized bf16 in hs tiles.

            Per-token stats over the partition(+tile) dim via ones-stationary
            matmuls; rstd = exp(-0.5*ln(var+eps)) so the ACT table set
            (natural_log_exp) is shared with the attention exp / softplus.
            """
            # cast to bf16 (also the tensor the stats are computed from)
            for k in range(NCt):
                nc.vector.tensor_scalar(hs[k][:], xs[k][:], 0.0, None, ALU.add)
            pm0 = pp1.tile([1, 512], F32, tag="p", name="pm0")
            pm1 = pp1.tile([1, 512], F32, tag="p", name="pm1")
            pv0 = pp1.tile([1, 512], F32, tag="p", name="pv0")
            pv1 = pp1.tile([1, 512], F32, tag="p", name="pv1")
            for k in range(NCt):
                sq = pmisc.tile([128, TOK], BF16, tag="sq", name="sq")
                nc.gpsimd.tensor_tensor(sq[:], hs[k][:], hs[k][:], ALU.mult)
                st, sp = (k == 0), (k == NCt - 1)
                nc.tensor.matmul(pm0[:], ones_sb[:], hs[k][:, 0:512],
                                 start=st, stop=sp)
                nc.tensor.matmul(pm1[:], ones_sb[:], hs[k][:, 512:1024],
                                 start=st, stop=sp)
                nc.tensor.matmul(pv0[:], ones_sb[:], sq[:, 0:512],
                                 start=st, stop=sp)
                nc.tensor.matmul(pv1[:], ones_sb[:], sq[:, 512:1024],
                                 start=st, stop=sp)
            invC = 1.0 / C
            # stats rows (32-aligned partitions): 0=mean, 32=mean^2,
            # 64=var->ln(var+eps), 96=rstd
            stt = pmisc.tile([128, TOK], F32, tag="stats", name="stt")
            nc.vector.tensor_scalar(stt[0:1, 0:512], pm0[:], invC, None, ALU.mult)
            nc.vector.tensor_scalar(stt[0:1, 512:1024], pm1[:], invC, None, ALU.mult)
            nc.vector.tensor_tensor(stt[32:33, :], stt[0:1, :], stt[0:1, :],
                                    ALU.mult)
            nc.vector.scalar_tensor_tensor(stt[64:65, 0:512], pv0[:], invC,
                                           stt[32:33, 0:512], ALU.mult,
                                           ALU.subtract)
            nc.vector.scalar_tensor_tensor(stt[64:65, 512:1024], pv1[:], invC,
                                           stt[32:33, 512:1024], ALU.mult,
                                           ALU.subtract)
            nc.scalar.activation(stt[64:65, :], stt[64:65, :], AF.Ln,
                                 bias=eps_t[:])
            nc.scalar.activation(stt[96:97, :], stt[64:65, :], AF.Exp, scale=-0.5)
            # partition_broadcast requires its source at partition 0 of the
            # tile, so mean/rstd are packed side by side in one [1, 2*TOK] row
            b16 = pb16.tile([1, 2 * TOK], BF16, tag="b16", name="b16")
            nc.vector.tensor_scalar(b16[0:1, 0:TOK], stt[0:1, :], 0.0, None,
                                    ALU.add)
            nc.vector.tensor_scalar(b16[0:1, TOK:2 * TOK], stt[96:97, :], 0.0,
                                    None, ALU.add)
            m_b = pmisc.tile([128, TOK], BF16, tag="m_b", name="m_b")
            r_b = pmisc.tile([128, TOK], BF16, tag="r_b", name="r_b")
            nc.gpsimd.partition_broadcast(m_b[:], b16[0:1, 0:TOK])
            nc.gpsimd.partition_broadcast(r_b[:], b16[0:1, TOK:2 * TOK])
            if dbg and not dbg_t.get("_stt_done"):
                nc.sync.dma_start(dbg_t["stt"].ap(), stt[:])
                nc.sync.dma_start(dbg_t["mb"].ap(), m_b[:])
                nc.sync.dma_start(dbg_t["rb"].ap(), r_b[:])
                dbg_t["_stt_done"] = True
            for k in range(NCt):
                nc.vector.tensor_tensor(hs[k][:], hs[k][:], m_b[:], ALU.subtract)
                nc.vector.tensor_tensor(hs[k][:], hs[k][:], r_b[:], ALU.mult)

        def dump(name, tiles, rows=128):
            if not dbg or name not in dbg_t:
                return
            ap = dbg_t[name].ap()
            for i, tl in enumerate(tiles):
                nc.sync.dma_start(ap[i * rows:(i + 1) * rows, :], tl[:])

        def layer(l):
            ln_transposed()
            if l == 0:
                dump("hs1", [hs[k][:] for k in range(NCt)])

            # ---- Q/K projections (W-stationary -> transposed out) ----
            for m in range(NCt):
                wq_m = pwqk.tile([128, NCt, 128], BF16, tag="wq", name="wq_m")
                wk_m = pwqk.tile([128, NCt, 128], BF16, tag="wk", name="wk_m")
                nc.sync.dma_start(wq_m[:], wq.ap()[l, m])
                nc.sync.dma_start(wk_m[:], wk.ap()[l, m])
                pq = pp2.tile([128, TOK], F32, tag="pp", name="pq")
                pk1 = pp1.tile([128, 512], F32, tag="p", name="pk1")
                pk2 = pp1.tile([128, 512], F32, tag="p", name="pk2")
                for k in range(NCt):
                    st, sp = (k == 0), (k == NCt - 1)
                    nc.tensor.matmul(pq[:, 0:512], wq_m[:, k, :],
                                     hs[k][:, 0:512], start=st, stop=sp)
                    nc.tensor.matmul(pq[:, 512:1024], wq_m[:, k, :],
                                     hs[k][:, 512:1024], start=st, stop=sp)
                    nc.tensor.matmul(pk1[:], wk_m[:, k, :],
                                     hs[k][:, 0:512], start=st, stop=sp)
                    nc.tensor.matmul(pk2[:], wk_m[:, k, :],
                                     hs[k][:, 512:1024], start=st, stop=sp)
                bqc = bq_sb[:, l * NCt + m: l * NCt + m + 1]
                bkc = bk_sb[:, l * NCt + m: l * NCt + m + 1]
                nc.vector.tensor_scalar(qT[m][:], pq[:], bqc, None, ALU.add)
                nc.vector.tensor_scalar(kT[m][:, 0:512], pk1[:], bkc, None, ALU.add)
                nc.vector.tensor_scalar(kT[m][:, 512:1024], pk2[:], bkc, None, ALU.add)

            if l == 0:
                dump("qT", [qT[m][:] for m in range(NCt)])
                dump("kT", [kT[m][:] for m in range(NCt)])

            # ---- V projection (hT-stationary -> natural v) ----
            wv_k = []
            for k in range(NCt):
                wvt = pwv.tile([128, C], BF16, tag=f"wv{k}", name=f"wv{k}")
                nc.sync.dma_start(wvt[:], wv.ap()[l, k])
                wv_k.append(wvt)
            bvl = pmisc.tile([1, C], BF16, tag="bvl", name="bvl")
            nc.sync.dma_start(bvl[:], bv.ap()[l:l + 1, :])
            bv_rep = pmisc.tile([128, C], BF16, tag="bv_rep", name="bv_rep")
            nc.gpsimd.partition_broadcast(bv_rep[:], bvl[:])
            for t in range(NT):
                pvp = pp2.tile([128, TOK], F32, tag="pp", name="pvp")
                tsl = slice(t * 128, (t + 1) * 128)
                for k in range(NCt):
                    st, sp = (k == 0), (k == NCt - 1)
                    nc.tensor.matmul(pvp[:, 0:512], hs[k][:, tsl],
                                     wv_k[k][:, 0:512], start=st, stop=sp)
                    nc.tensor.matmul(pvp[:, 512:1024], hs[k][:, tsl],
                                     wv_k[k][:, 512:1024], start=st, stop=sp)
                nc.vector.tensor_tensor(
                    vaug[t][:, :, 0:D],
                    pvp[:].rearrange("p (h d) -> p h d", h=H),
                    bv_rep[:].rearrange("p (h d) -> p h d", h=H),
                    ALU.add)

            if l == 0:
                dump("va", [vaug[t][:].rearrange("p h d -> p (h d)")
                            for t in range(NT)])

            # ---- attention (per head pair; row-group-concurrent scores) ----
            for hp in range(NCt):
                h0, h1 = 2 * hp, 2 * hp + 1
                stiles = []
                for tkt in range(4):
                    ps0 = pp2.tile([128, TOK], F32, tag="pp", name="ps0")
                    ps1 = pp2.tile([128, TOK], F32, tag="pp", name="ps1")
                    for b in range(2):
                        qsl = slice(b * 512, (b + 1) * 512)
                        ksl = slice(b * 512 + tkt * 128, b * 512 + (tkt + 1) * 128)
                        nc.tensor.matmul(ps0[:, qsl], kT[hp][0:64, ksl],
                                         qT[hp][0:64, qsl], start=True, stop=True)
                        nc.tensor.matmul(ps1[:, qsl], kT[hp][64:128, ksl],
                                         qT[hp][64:128, qsl], start=True, stop=True)
                    s0 = sslot[tkt]
                    s1 = sslot[4 + tkt]
                    nc.scalar.activation(s0[:], ps0[:], AF.Exp)
                    nc.scalar.activation(s1[:], ps1[:], AF.Exp)
                    if dbg and l == 0 and hp == 0 and tkt == 0:
                        nc.sync.dma_start(dbg_t["s00"].ap(), s0[:])
                        nc.sync.dma_start(dbg_t["s10"].ap(), s1[:])
                    stiles.append((s0, s1))
                for hi, h in enumerate((h0, h1)):
                    r0 = 64 * hi
                    pyA = pp1.tile([65, 512], F32, tag="p", name="pyA")
                    pyB = pp1.tile([65, 512], F32, tag="p", name="pyB")
                    for b, py in ((0, pyA), (1, pyB)):
                        bsl = slice(b * 512, (b + 1) * 512)
                        for tkt in range(4):
                            nc.tensor.matmul(py[:], vaug[b * 4 + tkt][:, h, :],
                                             stiles[tkt][hi][:, bsl],
                                             start=(tkt == 0), stop=(tkt == 3))
                    if dbg and l == 0 and hp == 0 and hi == 0:
                        pycp = pmisc.tile([128, TOK], F32, tag="stats",
                                          name="pycp")
                        nc.vector.tensor_scalar(pycp[0:65, 0:512], pyA[:],
                                                0.0, None, ALU.add)
                        nc.vector.tensor_scalar(pycp[0:65, 512:1024], pyB[:],
                                                0.0, None, ALU.add)
                        nc.sync.dma_start(dbg_t["py0"].ap(), pycp[0:65, :])
                    # reciprocal_approx_fast (custom DVE op) ignores AP
                    # partition bases: stage the denominator rows at
                    # partition 0 first, then run it base-0 -> base-0.
                    dn = pmisc.tile([128, TOK], F32, tag="stats", name="dn")
                    nc.vector.tensor_scalar(dn[0:1, 0:512], pyA[64:65, :],
                                            0.0, None, ALU.add)
                    nc.vector.tensor_scalar(dn[0:1, 512:1024], pyB[64:65, :],
                                            0.0, None, ALU.add)
                    rz = pmisc.tile([128, TOK], F32, tag="stats", name="rz")
                    nc.vector.reciprocal_approx_fast(rz[0:1, :], dn[0:1, :])
                    rz16 = pb16.tile([1, 2 * TOK], BF16, tag="b16", name="rz16")
                    nc.gpsimd.tensor_scalar(rz16[0:1, 0:TOK], rz[0:1, :], 0.0,
                                            None, ALU.add)
                    rz_rep = pr.tile([64, TOK], BF16, tag="rzr", name="rz_rep")
                    nc.gpsimd.partition_broadcast(rz_rep[:], rz16[0:1, 0:TOK])
                    if dbg and l == 0 and hp == 0 and hi == 0:
                        nc.sync.dma_start(dbg_t["rz0"].ap(), rz[0:1, :])
                        nc.sync.dma_start(dbg_t["rzr0"].ap(), rz_rep[:])
                    nc.vector.tensor_tensor(yT[hp][r0:r0 + 64, 0:512],
                                            pyA[0:64, :], rz_rep[:, 0:512], ALU.mult)
                    nc.vector.tensor_tensor(yT[hp][r0:r0 + 64, 512:1024],
                                            pyB[0:64, :], rz_rep[:, 512:1024], ALU.mult)

            if l == 0:
                dump("yT", [yT[m][:] for m in range(NCt)])

            # ---- O projection (W-stationary -> transposed, fused residual) ----
            for m in range(NCt):
                wo_m = pwo.tile([128, NCt, 128], BF16, tag="wo", name="wo_m")
                nc.sync.dma_start(wo_m[:], wo.ap()[l, m])
                po = pp2.tile([128, TOK], F32, tag="pp", name="po")
                for k in range(NCt):
                    st, sp = (k == 0), (k == NCt - 1)
                    nc.tensor.matmul(po[:, 0:512], wo_m[:, k, :],
                                     yT[k][:, 0:512], start=st, stop=sp)
                    nc.tensor.matmul(po[:, 512:1024], wo_m[:, k, :],
                                     yT[k][:, 512:1024], start=st, stop=sp)
                boc = bo_sb[:, l * NCt + m: l * NCt + m + 1]
                nc.vector.scalar_tensor_tensor(xs[m][:], po[:], boc, xs[m][:],
                                               ALU.add, ALU.add)

            if l == 0:
                dump("xo", [xs[m][:] for m in range(NCt)])

            ln_transposed()

            # ---- W1 + gelu (W-stationary -> transposed gT) ----
            for ft in range(NFt):
                w1_f = pw1.tile([128, NCt, 128], BF16, tag="w1", name="w1_f")
                nc.sync.dma_start(w1_f[:], w1.ap()[l, ft])
                pg = pp2.tile([128, TOK], F32, tag="pp", name="pg")
                for k in range(NCt):
                    st, sp = (k == 0), (k == NCt - 1)
                    nc.tensor.matmul(pg[:, 0:512], w1_f[:, k, :],
                                     hs[k][:, 0:512], start=st, stop=sp)
                    nc.tensor.matmul(pg[:, 512:1024], w1_f[:, k, :],
                                     hs[k][:, 512:1024], start=st, stop=sp)
                b1c = b1_sb[:, l * NFt + ft: l * NFt + ft + 1]
                nc.scalar.activation(big[ft][:], pg[:], AF.Gelu, bias=b1c)
            if l == 0:
                dump("gT", [big[ft][:] for ft in range(NFt)])

            # ---- W2 (W-stationary -> transposed, fused residual) ----
            for m in range(NCt):
                pw = pp2.tile([128, TOK], F32, tag="pp", name="pw")
                for half in range(2):
                    w2_m = pw2.tile([128, NFt // 2, 128], BF16, tag="w2",
                                    name="w2_m")
                    nc.sync.dma_start(
                        w2_m[:], w2.ap()[l, m][:, half * 16:(half + 1) * 16, :])
                    for kk in range(NFt // 2):
                        k = half * 16 + kk
                        st, sp = (k == 0), (k == NFt - 1)
                        nc.tensor.matmul(pw[:, 0:512], w2_m[:, kk, :],
                                         big[k][:, 0:512], start=st, stop=sp)
                        nc.tensor.matmul(pw[:, 512:1024], w2_m[:, kk, :],
                                         big[k][:, 512:1024], start=st, stop=sp)
                b2c = b2_sb[:, l * NCt + m: l * NCt + m + 1]
                nc.vector.scalar_tensor_tensor(xs[m][:], pw[:], b2c, xs[m][:],
                                               ALU.add, ALU.add)
            if l == 0:
                dump("xl", [xs[m][:] for m in range(NCt)])

        def whole_net(_iv=None):
            for t in range(NCt):
                nc.sync.dma_start(xs[t][:], x0.ap()[t * 128:(t + 1) * 128, :])
            for l in range(layers):
                layer(l)
            ln_transposed()
            yt = pmisc.tile([128, TOK], F32, tag="stats", name="yt")
            for tb in range(2):
                sl = slice(tb * 512, (tb + 1) * 512)
                ph = pp1.tile([1, 512], F32, tag="p", name="ph")
                for k in range(NCt):
                    nc.tensor.matmul(ph[:], hw_sb[:, k:k + 1], hs[k][:, sl],
                                     start=(k == 0), stop=(k == NCt - 1))
                # softplus(z) = ln(1 + exp(z)); z is bounded (~|z|<6) here
                nc.scalar.activation(yt[32:33, sl], ph[:], AF.Exp, bias=hb_sb[:])
                nc.vector.tensor_scalar(yt[32:33, sl], yt[32:33, sl], 1.0, None,
                                        ALU.add)
                nc.scalar.activation(yt[0:1, sl], yt[32:33, sl], AF.Ln)
            nc.sync.dma_start(yo.ap(), yt[0:1, :])

        if repeat == 1:
            whole_net()
        else:
            tc.For_i_unrolled(0, repeat, 1, whole_net, max_unroll=1)

    nc.compile()
    return nc


def prep_weights(inputs, layers=L):
    bf16 = ml_dtypes.bfloat16
    f32 = np.float32

    ln1_w, ln1_b = np.asarray(inputs["ln1_w"], f32), np.asarray(inputs["ln1_b"], f32)
    ln2_w, ln2_b = np.asarray(inputs["ln2_w"], f32), np.asarray(inputs["ln2_b"], f32)

    def pack_stat(w):  # [C_in, M] -> [M/128, 128(ci), C_in/128, 128(co)]
        ci, m = w.shape
        return w.reshape(ci // 128, 128, m // 128, 128).transpose(2, 1, 0, 3)

    scale = f32(1.0) / np.sqrt(f32(D))
    wq_t = np.empty((layers, NCt, 128, NCt, 128), bf16)
    wk_t = np.empty((layers, NCt, 128, NCt, 128), bf16)
    wv_t = np.empty((layers, NCt, 128, C), bf16)
    wo_t = np.empty((layers, NCt, 128, NCt, 128), bf16)
    w1_t = np.empty((layers, NFt, 128, NCt, 128), bf16)
    w2_t = np.empty((layers, NCt, 128, NFt, 128), bf16)
    bq_t = np.empty((128, layers * NCt), f32)
    bk_t = np.empty((128, layers * NCt), f32)
    bo_t = np.empty((128, layers * NCt), f32)
    b2_t = np.empty((128, layers * NCt), f32)
    b1_t = np.empty((128, layers * NFt), f32)
    bv_t = np.empty((layers, C), f32)

    for l in range(layers):
        Wq = np.asarray(inputs["Wq"][l], f32)
        Wk = np.asarray(inputs["Wk"][l], f32)
        Wv = np.asarray(inputs["Wv"][l], f32)
        Wo = np.asarray(inputs["Wo"][l], f32)
        W1 = np.asarray(inputs["W1"][l], f32)
        W2 = np.asarray(inputs["W2"][l], f32)
        d1 = ln1_w[l][:, None]
        d2 = ln2_w[l][:, None]
        wq_t[l] = pack_stat((d1 * Wq) * scale)
        wk_t[l] = pack_stat(d1 * Wk)
        wv_t[l] = (d1 * Wv).reshape(NCt, 128, C)
        wo_t[l] = pack_stat(Wo)
        w1_t[l] = pack_stat(d2 * W1)
        w2_t[l] = pack_stat(W2)
        bq_t[:, l * NCt:(l + 1) * NCt] = (
            (ln1_b[l] @ Wq + np.asarray(inputs["bq"][l], f32)) * scale
        ).reshape(NCt, 128).T
        bk_t[:, l * NCt:(l + 1) * NCt] = (
            ln1_b[l] @ Wk + np.asarray(inputs["bk"][l], f32)).reshape(NCt, 128).T
        bv_t[l] = ln1_b[l] @ Wv + np.asarray(inputs["bv"][l], f32)
        bo_t[:, l * NCt:(l + 1) * NCt] = np.asarray(
            inputs["bo"][l], f32).reshape(NCt, 128).T
        b1_t[:, l * NFt:(l + 1) * NFt] = (
            ln2_b[l] @ W1 + np.asarray(inputs["b1"][l], f32)).reshape(NFt, 128).T
        b2_t[:, l * NCt:(l + 1) * NCt] = np.asarray(
            inputs["b2"][l], f32).reshape(NCt, 128).T

    head_w = np.asarray(inputs["head_w"], f32)
    hw_f = np.asarray(inputs["ln_f_w"], f32)[:, None] * head_w  # [C,1]
    hb_f = (np.asarray(inputs["ln_f_b"], f32) @ head_w
            + np.asarray(inputs["head_b"], f32))

    out = {
        "wq": np.ascontiguousarray(wq_t), "wk": np.ascontiguousarray(wk_t),
        "wv": np.ascontiguousarray(wv_t), "wo": np.ascontiguousarray(wo_t),
        "w1": np.ascontiguousarray(w1_t), "w2": np.ascontiguousarray(w2_t),
        "bq": bq_t, "bk": bk_t, "bv": bv_t.astype(bf16),
        "bo": bo_t, "b1": b1_t, "b2": b2_t,
        "hw": np.ascontiguousarray(hw_f.reshape(NCt, 128).T.astype(bf16)),
        "hb": hb_f.reshape(1, 1),
    }
    return out


def prep_x0(inputs):
    idx = np.asarray(inputs["idx"])
    tok = np.asarray(inputs["tok_emb"], np.float32)
    pos = np.asarray(inputs["pos_emb"], np.float32)
    x0 = tok[idx] + pos  # [B, T, C]
    return [np.ascontiguousarray(x0[2 * c:2 * c + 2].reshape(TOK, C).T)
            for c in range(NCORES)]


class SpmdRunner:
    """Executes a compiled Bass module on the 8 axon-attached NeuronCores via
    PJRT (modeled on concourse.bass2jax.run_bass_via_pjrt, but jits once and
    keeps inputs device-resident so repeated calls are cheap)."""

    def __init__(self, nc, n_cores=NCORES):
        import jax
        from jax.sharding import Mesh, PartitionSpec
        from jax.experimental.shard_map import shard_map
        from concourse import bass2jax
        from concourse.bass2jax import _bass_exec_p, install_neuronx_cc_hook

        install_neuronx_cc_hook()
        self.jax = jax
        self.nc = nc
        self.n_cores = n_cores
        self.PartitionSpec = PartitionSpec

        partition_name = (
            nc.partition_id_tensor.name if nc.partition_id_tensor else None)
        in_names, out_names, out_avals = [], [], []
        self.extra_zero_names = []
        for alloc in nc.m.functions[0].allocations:
            if not isinstance(alloc, mybir.MemoryLocationSet):
                continue
            name = alloc.memorylocations[0].name
            if alloc.kind == "ExternalInput":
                if name != partition_name:
                    in_names.append(name)
            elif alloc.kind == "ExternalOutput":
                out_names.append(name)
                out_avals.append(jax.core.ShapedArray(
                    tuple(alloc.tensor_shape), mybir.dt.np(alloc.dtype)))
        if nc.dbg_addr is not None:
            self.extra_zero_names.append(nc.dbg_addr.name)

        self.in_names = list(in_names)
        self.out_names = out_names
        self.out_avals = out_avals
        n_params = len(in_names) + len(self.extra_zero_names)
        n_outs = len(out_avals)
        all_in_names = list(in_names) + self.extra_zero_names + list(out_names)
        if partition_name is not None:
            all_in_names.append(partition_name)

        def _body(*args):
            operands = list(args)
            if partition_name is not None:
                operands.append(bass2jax.partition_id_tensor())
            outs = _bass_exec_p.bind(
                *operands,
                out_avals=tuple(out_avals),
                in_names=tuple(all_in_names),
                out_names=tuple(out_names),
                lowering_input_output_aliases=(),
                sim_require_finite=True,
                sim_require_nnan=True,
                nc=nc,
            )
            return tuple(outs)

        devices = jax.devices()[:n_cores]
        assert len(devices) == n_cores, (
            f"need {n_cores} neuron cores, found {len(devices)}")
        self.mesh = Mesh(np.asarray(devices), ("core",))
        in_specs = (PartitionSpec("core"),) * (n_params + n_outs)
        out_specs = (PartitionSpec("core"),) * n_outs
        self.fn = jax.jit(
            shard_map(_body, mesh=self.mesh, in_specs=in_specs,
                      out_specs=out_specs, check_rep=False),
            keep_unused=True)
        self._dev_args = None

    def place_inputs(self, in_maps):
        jax = self.jax
        sharding = jax.sharding.NamedSharding(
            self.mesh, self.PartitionSpec("core"))
        args = []
        for name in self.in_names:
            concat = np.concatenate(
                [np.asarray(in_maps[c][name]) for c in range(self.n_cores)],
                axis=0)
            args.append(jax.device_put(concat, sharding))
        for name in self.extra_zero_names:
            args.append(jax.device_put(
                np.zeros((self.n_cores, 2), np.uint32), sharding))
        for aval in self.out_avals:
            args.append(jax.device_put(
                np.zeros((self.n_cores * aval.shape[0], *aval.shape[1:]),
                         aval.dtype), sharding))
        self._dev_args = args

    def run(self):
        outs = self.fn(*self._dev_args)
        self.jax.block_until_ready(outs)
        return outs

    def results(self, outs):
        per_core = []
        for c in range(self.n_cores):
            d = {}
            for i, name in enumerate(self.out_names):
                aval = self.out_avals[i]
                d[name] = np.asarray(outs[i]).reshape(
                    self.n_cores, *aval.shape)[c]
            per_core.append(d)
        return per_core


_CACHE = {}


def _get_runner(repeat=1):
    key = ("prog", repeat)
    if key not in _CACHE:
        ncb = build_program(layers=L, repeat=repeat)
        _CACHE[key] = SpmdRunner(ncb, NCORES)
    return _CACHE[key]


def kernel(**inputs) -> np.ndarray:
    w = prep_weights(inputs)
    x0s = prep_x0(inputs)
    runner = _get_runner(repeat=1)
    in_maps = [dict(w, x0=x0s[c]) for c in range(NCORES)]
    runner.place_inputs(in_maps)
    outs = runner.run()
    res = runner.results(outs)
    y = np.stack([res[c]["y"].reshape(2, 512) for c in range(NCORES)])
    return np.ascontiguousarray(y.reshape(16, 512).astype(np.float32))


# revision 35
# speedup vs baseline: 1.5947x; 1.5683x over previous
"""Trainium2 Bass kernel for a 12-layer dense transformer encoder
(B=16, T=512, C=1024, H=16, F=4096, V=30522), data-parallel over batch
across 8 NeuronCores (2 sequences per core).

kernel(**inputs) takes the FULL unsharded inputs (as produced by the
reference setup_inputs()) and returns the FULL [16, 512] float32 output.

Host side: embedding gather + positional add + transpose, LayerNorm weight
folding into the adjacent matmul weights (exact for any parametrization),
weight pre-tiling and bf16 cast.

Device side (identical SPMD program on all 8 cores): the residual stream is
kept TRANSPOSED (xT [C, tok]) on SBUF for the whole network, so no on-chip
transposes are ever needed:
  - LayerNorm stats (per-token mean/var over C) via ones-stationary matmuls
    on the TensorEngine, normalization applied with partition-broadcast
    mean/rstd rows; rstd computed as exp(-0.5*ln(var+eps)) so the ACT table
    set (natural_log_exp) is shared with the attention exp and the final
    softplus -- no table-set thrash.
  - Q/K/O/W1/W2 are weight-stationary matmuls producing transposed outputs
    directly; biases become per-partition scalars fused into the PSUM
    evacuation op.  V is hT-stationary producing natural v for the AV
    matmuls (ones-row augmented for the softmax denominator).
  - Attention exp on 2-bank [128,1024] PSUM pairs (one ACT op for both
    sequences), softmax normalization via reciprocal_approx_fast.
"""
import numpy as np
from contextlib import ExitStack

import ml_dtypes

import concourse.bass as bass
import concourse.mybir as mybir
import concourse.tile as tile
from concourse import bacc

F32 = mybir.dt.float32
BF16 = mybir.dt.bfloat16
AF = mybir.ActivationFunctionType
ALU = mybir.AluOpType

TOK, C, H, D, FF = 1024, 1024, 16, 64, 4096
NT = TOK // 128    # tok tiles
NCt = C // 128     # channel tiles
NFt = FF // 128    # ff tiles
EPS = 1e-5
NCORES = 8
L = 12


def build_program(layers=L, repeat=1, dbg=False):
    nc = bacc.Bacc("TRN2", target_bir_lowering=False, debug=False)
    LL = layers
    dbg_t = {}
    if dbg:
        dbg_t["hs1"] = nc.dram_tensor("dbg_hs1", (C, TOK), BF16, kind="ExternalOutput")
        dbg_t["qT"] = nc.dram_tensor("dbg_qT", (C, TOK), BF16, kind="ExternalOutput")
        dbg_t["kT"] = nc.dram_tensor("dbg_kT", (C, TOK), BF16, kind="ExternalOutput")
        dbg_t["va"] = nc.dram_tensor("dbg_va", (TOK, H * (D + 1)), BF16, kind="ExternalOutput")
        dbg_t["yT"] = nc.dram_tensor("dbg_yT", (C, TOK), BF16, kind="ExternalOutput")
        dbg_t["xo"] = nc.dram_tensor("dbg_xo", (C, TOK), F32, kind="ExternalOutput")
        dbg_t["gT"] = nc.dram_tensor("dbg_gT", (FF, TOK), BF16, kind="ExternalOutput")
        dbg_t["xl"] = nc.dram_tensor("dbg_xl", (C, TOK), F32, kind="ExternalOutput")
        dbg_t["stt"] = nc.dram_tensor("dbg_stt", (128, TOK), F32, kind="ExternalOutput")
        dbg_t["mb"] = nc.dram_tensor("dbg_mb", (128, TOK), BF16, kind="ExternalOutput")
        dbg_t["rb"] = nc.dram_tensor("dbg_rb", (128, TOK), BF16, kind="ExternalOutput")
        dbg_t["s00"] = nc.dram_tensor("dbg_s00", (128, TOK), BF16, kind="ExternalOutput")
        dbg_t["s10"] = nc.dram_tensor("dbg_s10", (128, TOK), BF16, kind="ExternalOutput")
        dbg_t["py0"] = nc.dram_tensor("dbg_py0", (65, TOK), F32, kind="ExternalOutput")
        dbg_t["rz0"] = nc.dram_tensor("dbg_rz0", (1, TOK), F32, kind="ExternalOutput")
        dbg_t["rzr0"] = nc.dram_tensor("dbg_rzr0", (64, TOK), BF16, kind="ExternalOutput")

    x0 = nc.dram_tensor("x0", (C, TOK), F32, kind="ExternalInput")
    wq = nc.dram_tensor("wq", (LL, NCt, 128, NCt, 128), BF16, kind="ExternalInput")
    wk = nc.dram_tensor("wk", (LL, NCt, 128, NCt, 128), BF16, kind="ExternalInput")
    wv = nc.dram_tensor("wv", (LL, NCt, 128, C), BF16, kind="ExternalInput")
    wo = nc.dram_tensor("wo", (LL, NCt, 128, NCt, 128), BF16, kind="ExternalInput")
    w1 = nc.dram_tensor("w1", (LL, NFt, 128, NCt, 128), BF16, kind="ExternalInput")
    w2 = nc.dram_tensor("w2", (LL, NCt, 128, NFt, 128), BF16, kind="ExternalInput")
    bq = nc.dram_tensor("bq", (128, LL * NCt), F32, kind="ExternalInput")
    bk = nc.dram_tensor("bk", (128, LL * NCt), F32, kind="ExternalInput")
    bv = nc.dram_tensor("bv", (LL, C), BF16, kind="ExternalInput")
    bo = nc.dram_tensor("bo", (128, LL * NCt), F32, kind="ExternalInput")
    b1 = nc.dram_tensor("b1", (128, LL * NFt), F32, kind="ExternalInput")
    b2 = nc.dram_tensor("b2", (128, LL * NCt), F32, kind="ExternalInput")
    hw = nc.dram_tensor("hw", (128, NCt), BF16, kind="ExternalInput")
    hb = nc.dram_tensor("hb", (1, 1), F32, kind="ExternalInput")
    yo = nc.dram_tensor("y", (1, TOK), F32, kind="ExternalOutput")

    with tile.TileContext(nc) as tc, ExitStack() as ctx:
        px = ctx.enter_context(tc.tile_pool(name="px", bufs=1))
        pwqk = ctx.enter_context(tc.tile_pool(name="pwqk", bufs=2))
        pwv = ctx.enter_context(tc.tile_pool(name="pwv", bufs=1))
        pwo = ctx.enter_context(tc.tile_pool(name="pwo", bufs=2))
        pw1 = ctx.enter_context(tc.tile_pool(name="pw1", bufs=2))
        pw2 = ctx.enter_context(tc.tile_pool(name="pw2", bufs=2))
        pbias = ctx.enter_context(tc.tile_pool(name="pbias", bufs=1))
        pmisc = ctx.enter_context(tc.tile_pool(name="pmisc", bufs=2))
        pr = ctx.enter_context(tc.tile_pool(name="pr", bufs=1))
        pb16 = ctx.enter_context(tc.tile_pool(name="pb16", bufs=1))
        pp1 = ctx.enter_context(tc.tile_pool(name="pp1", bufs=4, space="PSUM"))
        pp2 = ctx.enter_context(tc.tile_pool(name="pp2", bufs=2, space="PSUM"))

        # persistent SBUF tiles
        xs, hs, vaug, big = [], [], [], []
        for t in range(NCt):
            xs.append(px.tile([128, TOK], F32, tag=f"x{t}", name=f"x{t}"))
        for t in range(NCt):
            hs.append(px.tile([128, TOK], BF16, tag=f"h{t}", name=f"h{t}"))
        for t in range(NT):
            vaug.append(px.tile([128, H, D + 1], BF16, tag=f"v{t}", name=f"v{t}"))
        for i in range(32):
            big.append(px.tile([128, TOK], BF16, tag=f"big{i}", name=f"big{i}"))
        qT, kT, yT, sslot = big[0:8], big[8:16], big[16:24], big[24:32]

        ones_sb = pbias.tile([128, 1], BF16, tag="ones", name="ones_sb")
        nc.vector.memset(ones_sb[:], 1.0)
        eps_t = pbias.tile([1, 1], F32, tag="eps", name="eps_t")
        nc.vector.memset(eps_t[:], EPS)
        for t in range(NT):
            nc.vector.memset(vaug[t][:, :, D:D + 1], 1.0)

        # preload all biases
        bq_sb = pbias.tile([128, LL * NCt], F32, tag="bq", name="bq_sb")
        bk_sb = pbias.tile([128, LL * NCt], F32, tag="bk", name="bk_sb")
        bo_sb = pbias.tile([128, LL * NCt], F32, tag="bo", name="bo_sb")
        b2_sb = pbias.tile([128, LL * NCt], F32, tag="b2", name="b2_sb")
        b1_sb = pbias.tile([128, LL * NFt], F32, tag="b1", name="b1_sb")
        hw_sb = pbias.tile([128, NCt], BF16, tag="hw", name="hw_sb")
        hb_sb = pbias.tile([1, 1], F32, tag="hb", name="hb_sb")
        nc.sync.dma_start(bq_sb[:], bq.ap())
        nc.sync.dma_start(bk_sb[:], bk.ap())
        nc.sync.dma_start(bo_sb[:], bo.ap())
        nc.sync.dma_start(b2_sb[:], b2.ap())
        nc.sync.dma_start(b1_sb[:], b1.ap())
        nc.sync.dma_start(hw_sb[:], hw.ap())
        nc.sync.dma_start(hb_sb[:], hb.ap())

        def ln_transposed():
            """x (xs tiles, [C,tok] f32) -> normalized bf16 in hs tiles.

            Per-token stats over the partition(+tile) dim via ones-stationary
            matmuls; rstd = exp(-0.5*ln(var+eps)) so the ACT table set
            (natural_log_exp) is shared with the attention exp / softplus.
            """
            # cast to bf16 (also the tensor the stats are computed from)
            for k in range(NCt):
                nc.vector.tensor_scalar(hs[k][:], xs[k][:], 0.0, None, ALU.add)
            pm0 = pp1.tile([1, 512], F32, tag="p", name="pm0")
            pm1 = pp1.tile([1, 512], F32, tag="p", name="pm1")
            pv0 = pp1.tile([1, 512], F32, tag="p", name="pv0")
            pv1 = pp1.tile([1, 512], F32, tag="p", name="pv1")
            for k in range(NCt):
                sq = pmisc.tile([128, TOK], BF16, tag="sq", name="sq")
                nc.vector.tensor_tensor(sq[:], hs[k][:], hs[k][:], ALU.mult)
                st, sp = (k == 0), (k == NCt - 1)
                nc.tensor.matmul(pm0[:], ones_sb[:], hs[k][:, 0:512],
                                 start=st, stop=sp)
                nc.tensor.matmul(pm1[:], ones_sb[:], hs[k][:, 512:1024],
                                 start=st, stop=sp)
                nc.tensor.matmul(pv0[:], ones_sb[:], sq[:, 0:512],
                                 start=st, stop=sp)
                nc.tensor.matmul(pv1[:], ones_sb[:], sq[:, 512:1024],
                                 start=st, stop=sp)
            invC = 1.0 / C
            # stats rows (32-aligned partitions): 0=mean, 32=mean^2,
            # 64=var->ln(var+eps), 96=rstd
            stt = pmisc.tile([128, TOK], F32, tag="stats", name="stt")
            nc.vector.tensor_scalar(stt[0:1, 0:512], pm0[:], invC, None, ALU.mult)
            nc.vector.tensor_scalar(stt[0:1, 512:1024], pm1[:], invC, None, ALU.mult)
            nc.vector.tensor_tensor(stt[32:33, :], stt[0:1, :], stt[0:1, :],
                                    ALU.mult)
            nc.vector.scalar_tensor_tensor(stt[64:65, 0:512], pv0[:], invC,
                                           stt[32:33, 0:512], ALU.mult,
                                           ALU.subtract)
            nc.vector.scalar_tensor_tensor(stt[64:65, 512:1024], pv1[:], invC,
                                           stt[32:33, 512:1024], ALU.mult,
                                           ALU.subtract)
            nc.scalar.activation(stt[64:65, :], stt[64:65, :], AF.Ln,
                                 bias=eps_t[:])
            nc.scalar.activation(stt[96:97, :], stt[64:65, :], AF.Exp, scale=-0.5)
            # partition_broadcast requires its source at partition 0 of the
            # tile, so mean/rstd are packed side by side in one [1, 2*TOK] row
            b16 = pb16.tile([1, 2 * TOK], BF16, tag="b16", name="b16")
            nc.vector.tensor_scalar(b16[0:1, 0:TOK], stt[0:1, :], 0.0, None,
                                    ALU.add)
            nc.vector.tensor_scalar(b16[0:1, TOK:2 * TOK], stt[96:97, :], 0.0,
                                    None, ALU.add)
            m_b = pmisc.tile([128, TOK], BF16, tag="m_b", name="m_b")
            r_b = pmisc.tile([128, TOK], BF16, tag="r_b", name="r_b")
            nc.gpsimd.partition_broadcast(m_b[:], b16[0:1, 0:TOK])
            nc.gpsimd.partition_broadcast(r_b[:], b16[0:1, TOK:2 * TOK])
            if dbg and not dbg_t.get("_stt_done"):
                nc.sync.dma_start(dbg_t["stt"].ap(), stt[:])
                nc.sync.dma_start(dbg_t["mb"].ap(), m_b[:])
                nc.sync.dma_start(dbg_t["rb"].ap(), r_b[:])
                dbg_t["_stt_done"] = True
            for k in range(NCt):
                nc.vector.tensor_tensor(hs[k][:], hs[k][:], m_b[:], ALU.subtract)
                nc.vector.tensor_tensor(hs[k][:], hs[k][:], r_b[:], ALU.mult)

        def dump(name, tiles, rows=128):
            if not dbg or name not in dbg_t:
                return
            ap = dbg_t[name].ap()
            for i, tl in enumerate(tiles):
                nc.sync.dma_start(ap[i * rows:(i + 1) * rows, :], tl[:])

        def layer(l):
            ln_transposed()
            if l == 0:
                dump("hs1", [hs[k][:] for k in range(NCt)])

            # ---- Q/K projections (W-stationary -> transposed out) ----
            for m in range(NCt):
                wq_m = pwqk.tile([128, NCt, 128], BF16, tag="wq", name="wq_m")
                wk_m = pwqk.tile([128, NCt, 128], BF16, tag="wk", name="wk_m")
                nc.sync.dma_start(wq_m[:], wq.ap()[l, m])
                nc.sync.dma_start(wk_m[:], wk.ap()[l, m])
                pq = pp2.tile([128, TOK], F32, tag="pp", name="pq")
                pk1 = pp1.tile([128, 512], F32, tag="p", name="pk1")
                pk2 = pp1.tile([128, 512], F32, tag="p", name="pk2")
                for k in range(NCt):
                    st, sp = (k == 0), (k == NCt - 1)
                    nc.tensor.matmul(pq[:, 0:512], wq_m[:, k, :],
                                     hs[k][:, 0:512], start=st, stop=sp)
                    nc.tensor.matmul(pq[:, 512:1024], wq_m[:, k, :],
                                     hs[k][:, 512:1024], start=st, stop=sp)
                    nc.tensor.matmul(pk1[:], wk_m[:, k, :],
                                     hs[k][:, 0:512], start=st, stop=sp)
                    nc.tensor.matmul(pk2[:], wk_m[:, k, :],
                                     hs[k][:, 512:1024], start=st, stop=sp)
                bqc = bq_sb[:, l * NCt + m: l * NCt + m + 1]
                bkc = bk_sb[:, l * NCt + m: l * NCt + m + 1]
                nc.vector.tensor_scalar(qT[m][:], pq[:], bqc, None, ALU.add)
                nc.vector.tensor_scalar(kT[m][:, 0:512], pk1[:], bkc, None, ALU.add)
                nc.vector.tensor_scalar(kT[m][:, 512:1024], pk2[:], bkc, None, ALU.add)

            if l == 0:
                dump("qT", [qT[m][:] for m in range(NCt)])
                dump("kT", [kT[m][:] for m in range(NCt)])

            # ---- V projection (hT-stationary -> natural v) ----
            wv_k = []
            for k in range(NCt):
                wvt = pwv.tile([128, C], BF16, tag=f"wv{k}", name=f"wv{k}")
                nc.sync.dma_start(wvt[:], wv.ap()[l, k])
                wv_k.append(wvt)
            bvl = pmisc.tile([1, C], BF16, tag="bvl", name="bvl")
            nc.sync.dma_start(bvl[:], bv.ap()[l:l + 1, :])
            bv_rep = pmisc.tile([128, C], BF16, tag="bv_rep", name="bv_rep")
            nc.gpsimd.partition_broadcast(bv_rep[:], bvl[:])
            for t in range(NT):
                pvp = pp2.tile([128, TOK], F32, tag="pp", name="pvp")
                tsl = slice(t * 128, (t + 1) * 128)
                for k in range(NCt):
                    st, sp = (k == 0), (k == NCt - 1)
                    nc.tensor.matmul(pvp[:, 0:512], hs[k][:, tsl],
                                     wv_k[k][:, 0:512], start=st, stop=sp)
                    nc.tensor.matmul(pvp[:, 512:1024], hs[k][:, tsl],
                                     wv_k[k][:, 512:1024], start=st, stop=sp)
                nc.vector.tensor_tensor(
                    vaug[t][:, :, 0:D],
                    pvp[:].rearrange("p (h d) -> p h d", h=H),
                    bv_rep[:].rearrange("p (h d) -> p h d", h=H),
                    ALU.add)

            if l == 0:
                dump("va", [vaug[t][:].rearrange("p h d -> p (h d)")
                            for t in range(NT)])

            # ---- attention (per head pair; row-group-concurrent scores) ----
            for hp in range(NCt):
                h0, h1 = 2 * hp, 2 * hp + 1
                stiles = []
                for tkt in range(4):
                    ps0 = pp2.tile([128, TOK], F32, tag="pp", name="ps0")
                    ps1 = pp2.tile([128, TOK], F32, tag="pp", name="ps1")
                    for b in range(2):
                        qsl = slice(b * 512, (b + 1) * 512)
                        ksl = slice(b * 512 + tkt * 128, b * 512 + (tkt + 1) * 128)
                        nc.tensor.matmul(ps0[:, qsl], kT[hp][0:64, ksl],
                                         qT[hp][0:64, qsl], start=True, stop=True)
                        nc.tensor.matmul(ps1[:, qsl], kT[hp][64:128, ksl],
                                         qT[hp][64:128, qsl], start=True, stop=True)
                    s0 = sslot[tkt]
                    s1 = sslot[4 + tkt]
                    nc.scalar.activation(s0[:], ps0[:], AF.Exp)
                    nc.scalar.activation(s1[:], ps1[:], AF.Exp)
                    if dbg and l == 0 and hp == 0 and tkt == 0:
                        nc.sync.dma_start(dbg_t["s00"].ap(), s0[:])
                        nc.sync.dma_start(dbg_t["s10"].ap(), s1[:])
                    stiles.append((s0, s1))
                for hi, h in enumerate((h0, h1)):
                    r0 = 64 * hi
                    pyA = pp1.tile([65, 512], F32, tag="p", name="pyA")
                    pyB = pp1.tile([65, 512], F32, tag="p", name="pyB")
                    for b, py in ((0, pyA), (1, pyB)):
                        bsl = slice(b * 512, (b + 1) * 512)
                        for tkt in range(4):
                            nc.tensor.matmul(py[:], vaug[b * 4 + tkt][:, h, :],
                                             stiles[tkt][hi][:, bsl],
                                             start=(tkt == 0), stop=(tkt == 3))
                    if dbg and l == 0 and hp == 0 and hi == 0:
                        pycp = pmisc.tile([128, TOK], F32, tag="stats",
                                          name="pycp")
                        nc.vector.tensor_scalar(pycp[0:65, 0:512], pyA[:],
                                                0.0, None, ALU.add)
                        nc.vector.tensor_scalar(pycp[0:65, 512:1024], pyB[:],
                                                0.0, None, ALU.add)
                        nc.sync.dma_start(dbg_t["py0"].ap(), pycp[0:65, :])
                    # reciprocal_approx_fast (custom DVE op) ignores AP
                    # partition bases: stage the denominator rows at
                    # partition 0 first, then run it base-0 -> base-0.
                    dn = pmisc.tile([128, TOK], F32, tag="stats", name="dn")
                    nc.vector.tensor_scalar(dn[0:1, 0:512], pyA[64:65, :],
                                            0.0, None, ALU.add)
                    nc.vector.tensor_scalar(dn[0:1, 512:1024], pyB[64:65, :],
                                            0.0, None, ALU.add)
                    rz = pmisc.tile([128, TOK], F32, tag="stats", name="rz")
                    nc.vector.reciprocal_approx_fast(rz[0:1, :], dn[0:1, :])
                    rz16 = pb16.tile([1, 2 * TOK], BF16, tag="b16", name="rz16")
                    nc.vector.tensor_scalar(rz16[0:1, 0:TOK], rz[0:1, :], 0.0,
                                            None, ALU.add)
                    rz_rep = pr.tile([64, TOK], BF16, tag="rzr", name="rz_rep")
                    nc.gpsimd.partition_broadcast(rz_rep[:], rz16[0:1, 0:TOK])
                    if dbg and l == 0 and hp == 0 and hi == 0:
                        nc.sync.dma_start(dbg_t["rz0"].ap(), rz[0:1, :])
                        nc.sync.dma_start(dbg_t["rzr0"].ap(), rz_rep[:])
                    nc.vector.tensor_tensor(yT[hp][r0:r0 + 64, 0:512],
                                            pyA[0:64, :], rz_rep[:, 0:512], ALU.mult)
                    nc.vector.tensor_tensor(yT[hp][r0:r0 + 64, 512:1024],
                                            pyB[0:64, :], rz_rep[:, 512:1024], ALU.mult)

            if l == 0:
                dump("yT", [yT[m][:] for m in range(NCt)])

            # ---- O projection (W-stationary -> transposed, fused residual) ----
            for m in range(NCt):
                wo_m = pwo.tile([128, NCt, 128], BF16, tag="wo", name="wo_m")
                nc.sync.dma_start(wo_m[:], wo.ap()[l, m])
                po = pp2.tile([128, TOK], F32, tag="pp", name="po")
                for k in range(NCt):
                    st, sp = (k == 0), (k == NCt - 1)
                    nc.tensor.matmul(po[:, 0:512], wo_m[:, k, :],
                                     yT[k][:, 0:512], start=st, stop=sp)
                    nc.tensor.matmul(po[:, 512:1024], wo_m[:, k, :],
                                     yT[k][:, 512:1024], start=st, stop=sp)
                boc = bo_sb[:, l * NCt + m: l * NCt + m + 1]
                nc.vector.scalar_tensor_tensor(xs[m][:], po[:], boc, xs[m][:],
                                               ALU.add, ALU.add)

            if l == 0:
                dump("xo", [xs[m][:] for m in range(NCt)])

            ln_transposed()

            # ---- W1 + gelu (W-stationary -> transposed gT) ----
            for ft in range(NFt):
                w1_f = pw1.tile([128, NCt, 128], BF16, tag="w1", name="w1_f")
                nc.sync.dma_start(w1_f[:], w1.ap()[l, ft])
                pg = pp2.tile([128, TOK], F32, tag="pp", name="pg")
                for k in range(NCt):
                    st, sp = (k == 0), (k == NCt - 1)
                    nc.tensor.matmul(pg[:, 0:512], w1_f[:, k, :],
                                     hs[k][:, 0:512], start=st, stop=sp)
                    nc.tensor.matmul(pg[:, 512:1024], w1_f[:, k, :],
                                     hs[k][:, 512:1024], start=st, stop=sp)
                b1c = b1_sb[:, l * NFt + ft: l * NFt + ft + 1]
                nc.scalar.activation(big[ft][:], pg[:], AF.Gelu, bias=b1c)
            if l == 0:
                dump("gT", [big[ft][:] for ft in range(NFt)])

            # ---- W2 (W-stationary -> transposed, fused residual) ----
            for m in range(NCt):
                pw = pp2.tile([128, TOK], F32, tag="pp", name="pw")
                for half in range(2):
                    w2_m = pw2.tile([128, NFt // 2, 128], BF16, tag="w2",
                                    name="w2_m")
                    nc.sync.dma_start(
                        w2_m[:], w2.ap()[l, m][:, half * 16:(half + 1) * 16, :])
                    for kk in range(NFt // 2):
                        k = half * 16 + kk
                        st, sp = (k == 0), (k == NFt - 1)
                        nc.tensor.matmul(pw[:, 0:512], w2_m[:, kk, :],
                                         big[k][:, 0:512], start=st, stop=sp)
                        nc.tensor.matmul(pw[:, 512:1024], w2_m[:, kk, :],
                                         big[k][:, 512:1024], start=st, stop=sp)
                b2c = b2_sb[:, l * NCt + m: l * NCt + m + 1]
                nc.vector.scalar_tensor_tensor(xs[m][:], pw[:], b2c, xs[m][:],
                                               ALU.add, ALU.add)
            if l == 0:
                dump("xl", [xs[m][:] for m in range(NCt)])

        def whole_net(_iv=None):
            for t in range(NCt):
                nc.sync.dma_start(xs[t][:], x0.ap()[t * 128:(t + 1) * 128, :])
            for l in range(layers):
                layer(l)
            ln_transposed()
            yt = pmisc.tile([128, TOK], F32, tag="stats", name="yt")
            for tb in range(2):
                sl = slice(tb * 512, (tb + 1) * 512)
                ph = pp1.tile([1, 512], F32, tag="p", name="ph")
                for k in range(NCt):
                    nc.tensor.matmul(ph[:], hw_sb[:, k:k + 1], hs[k][:, sl],
                                     start=(k == 0), stop=(k == NCt - 1))
                # softplus(z) = ln(1 + exp(z)); z is bounded (~|z|<6) here
                nc.scalar.activation(yt[32:33, sl], ph[:], AF.Exp, bias=hb_sb[:])
                nc.vector.tensor_scalar(yt[32:33, sl], yt[32:33, sl], 1.0, None,
                                        ALU.add)
                nc.scalar.activation(yt[0:1, sl], yt[32:33, sl], AF.Ln)
            nc.sync.dma_start(yo.ap(), yt[0:1, :])

        if repeat == 1:
            whole_net()
        else:
            tc.For_i_unrolled(0, repeat, 1, whole_net, max_unroll=1)

    nc.compile()
    return nc


def prep_weights(inputs, layers=L):
    bf16 = ml_dtypes.bfloat16
    f32 = np.float32

    ln1_w, ln1_b = np.asarray(inputs["ln1_w"], f32), np.asarray(inputs["ln1_b"], f32)
    ln2_w, ln2_b = np.asarray(inputs["ln2_w"], f32), np.asarray(inputs["ln2_b"], f32)

    def pack_stat(w):  # [C_in, M] -> [M/128, 128(ci), C_in/128, 128(co)]
        ci, m = w.shape
        return w.reshape(ci // 128, 128, m // 128, 128).transpose(2, 1, 0, 3)

    scale = f32(1.0) / np.sqrt(f32(D))
    wq_t = np.empty((layers, NCt, 128, NCt, 128), bf16)
    wk_t = np.empty((layers, NCt, 128, NCt, 128), bf16)
    wv_t = np.empty((layers, NCt, 128, C), bf16)
    wo_t = np.empty((layers, NCt, 128, NCt, 128), bf16)
    w1_t = np.empty((layers, NFt, 128, NCt, 128), bf16)
    w2_t = np.empty((layers, NCt, 128, NFt, 128), bf16)
    bq_t = np.empty((128, layers * NCt), f32)
    bk_t = np.empty((128, layers * NCt), f32)
    bo_t = np.empty((128, layers * NCt), f32)
    b2_t = np.empty((128, layers * NCt), f32)
    b1_t = np.empty((128, layers * NFt), f32)
    bv_t = np.empty((layers, C), f32)

    for l in range(layers):
        Wq = np.asarray(inputs["Wq"][l], f32)
        Wk = np.asarray(inputs["Wk"][l], f32)
        Wv = np.asarray(inputs["Wv"][l], f32)
        Wo = np.asarray(inputs["Wo"][l], f32)
        W1 = np.asarray(inputs["W1"][l], f32)
        W2 = np.asarray(inputs["W2"][l], f32)
        d1 = ln1_w[l][:, None]
        d2 = ln2_w[l][:, None]
        wq_t[l] = pack_stat((d1 * Wq) * scale)
        wk_t[l] = pack_stat(d1 * Wk)
        wv_t[l] = (d1 * Wv).reshape(NCt, 128, C)
        wo_t[l] = pack_stat(Wo)
        w1_t[l] = pack_stat(d2 * W1)
        w2_t[l] = pack_stat(W2)
        bq_t[:, l * NCt:(l + 1) * NCt] = (
            (ln1_b[l] @ Wq + np.asarray(inputs["bq"][l], f32)) * scale
        ).reshape(NCt, 128).T
        bk_t[:, l * NCt:(l + 1) * NCt] = (
            ln1_b[l] @ Wk + np.asarray(inputs["bk"][l], f32)).reshape(NCt, 128).T
        bv_t[l] = ln1_b[l] @ Wv + np.asarray(inputs["bv"][l], f32)
        bo_t[:, l * NCt:(l + 1) * NCt] = np.asarray(
            inputs["bo"][l], f32).reshape(NCt, 128).T
        b1_t[:, l * NFt:(l + 1) * NFt] = (
            ln2_b[l] @ W1 + np.asarray(inputs["b1"][l], f32)).reshape(NFt, 128).T
        b2_t[:, l * NCt:(l + 1) * NCt] = np.asarray(
            inputs["b2"][l], f32).reshape(NCt, 128).T

    head_w = np.asarray(inputs["head_w"], f32)
    hw_f = np.asarray(inputs["ln_f_w"], f32)[:, None] * head_w  # [C,1]
    hb_f = (np.asarray(inputs["ln_f_b"], f32) @ head_w
            + np.asarray(inputs["head_b"], f32))

    out = {
        "wq": np.ascontiguousarray(wq_t), "wk": np.ascontiguousarray(wk_t),
        "wv": np.ascontiguousarray(wv_t), "wo": np.ascontiguousarray(wo_t),
        "w1": np.ascontiguousarray(w1_t), "w2": np.ascontiguousarray(w2_t),
        "bq": bq_t, "bk": bk_t, "bv": bv_t.astype(bf16),
        "bo": bo_t, "b1": b1_t, "b2": b2_t,
        "hw": np.ascontiguousarray(hw_f.reshape(NCt, 128).T.astype(bf16)),
        "hb": hb_f.reshape(1, 1),
    }
    return out


def prep_x0(inputs):
    idx = np.asarray(inputs["idx"])
    tok = np.asarray(inputs["tok_emb"], np.float32)
    pos = np.asarray(inputs["pos_emb"], np.float32)
    x0 = tok[idx] + pos  # [B, T, C]
    return [np.ascontiguousarray(x0[2 * c:2 * c + 2].reshape(TOK, C).T)
            for c in range(NCORES)]


class SpmdRunner:
    """Executes a compiled Bass module on the 8 axon-attached NeuronCores via
    PJRT (modeled on concourse.bass2jax.run_bass_via_pjrt, but jits once and
    keeps inputs device-resident so repeated calls are cheap)."""

    def __init__(self, nc, n_cores=NCORES):
        import jax
        from jax.sharding import Mesh, PartitionSpec
        from jax.experimental.shard_map import shard_map
        from concourse import bass2jax
        from concourse.bass2jax import _bass_exec_p, install_neuronx_cc_hook

        install_neuronx_cc_hook()
        self.jax = jax
        self.nc = nc
        self.n_cores = n_cores
        self.PartitionSpec = PartitionSpec

        partition_name = (
            nc.partition_id_tensor.name if nc.partition_id_tensor else None)
        in_names, out_names, out_avals = [], [], []
        self.extra_zero_names = []
        for alloc in nc.m.functions[0].allocations:
            if not isinstance(alloc, mybir.MemoryLocationSet):
                continue
            name = alloc.memorylocations[0].name
            if alloc.kind == "ExternalInput":
                if name != partition_name:
                    in_names.append(name)
            elif alloc.kind == "ExternalOutput":
                out_names.append(name)
                out_avals.append(jax.core.ShapedArray(
                    tuple(alloc.tensor_shape), mybir.dt.np(alloc.dtype)))
        if nc.dbg_addr is not None:
            self.extra_zero_names.append(nc.dbg_addr.name)

        self.in_names = list(in_names)
        self.out_names = out_names
        self.out_avals = out_avals
        n_params = len(in_names) + len(self.extra_zero_names)
        n_outs = len(out_avals)
        all_in_names = list(in_names) + self.extra_zero_names + list(out_names)
        if partition_name is not None:
            all_in_names.append(partition_name)

        def _body(*args):
            operands = list(args)
            if partition_name is not None:
                operands.append(bass2jax.partition_id_tensor())
            outs = _bass_exec_p.bind(
                *operands,
                out_avals=tuple(out_avals),
                in_names=tuple(all_in_names),
                out_names=tuple(out_names),
                lowering_input_output_aliases=(),
                sim_require_finite=True,
                sim_require_nnan=True,
                nc=nc,
            )
            return tuple(outs)

        devices = jax.devices()[:n_cores]
        assert len(devices) == n_cores, (
            f"need {n_cores} neuron cores, found {len(devices)}")
        self.mesh = Mesh(np.asarray(devices), ("core",))
        in_specs = (PartitionSpec("core"),) * (n_params + n_outs)
        out_specs = (PartitionSpec("core"),) * n_outs
        self.fn = jax.jit(
            shard_map(_body, mesh=self.mesh, in_specs=in_specs,
                      out_specs=out_specs, check_rep=False),
            keep_unused=True)
        self._dev_args = None

    def place_inputs(self, in_maps):
        jax = self.jax
        sharding = jax.sharding.NamedSharding(
            self.mesh, self.PartitionSpec("core"))
        args = []
        for name in self.in_names:
            concat = np.concatenate(
                [np.asarray(in_maps[c][name]) for c in range(self.n_cores)],
                axis=0)
            args.append(jax.device_put(concat, sharding))
        for name in self.extra_zero_names:
            args.append(jax.device_put(
                np.zeros((self.n_cores, 2), np.uint32), sharding))
        for aval in self.out_avals:
            args.append(jax.device_put(
                np.zeros((self.n_cores * aval.shape[0], *aval.shape[1:]),
                         aval.dtype), sharding))
        self._dev_args = args

    def run(self):
        outs = self.fn(*self._dev_args)
        self.jax.block_until_ready(outs)
        return outs

    def results(self, outs):
        per_core = []
        for c in range(self.n_cores):
            d = {}
            for i, name in enumerate(self.out_names):
                aval = self.out_avals[i]
                d[name] = np.asarray(outs[i]).reshape(
                    self.n_cores, *aval.shape)[c]
            per_core.append(d)
        return per_core


_CACHE = {}


def _get_runner(repeat=1):
    key = ("prog", repeat)
    if key not in _CACHE:
        ncb = build_program(layers=L, repeat=repeat)
        _CACHE[key] = SpmdRunner(ncb, NCORES)
    return _CACHE[key]


def kernel(**inputs) -> np.ndarray:
    w = prep_weights(inputs)
    x0s = prep_x0(inputs)
    runner = _get_runner(repeat=1)
    in_maps = [dict(w, x0=x0s[c]) for c in range(NCORES)]
    runner.place_inputs(in_maps)
    outs = runner.run()
    res = runner.results(outs)
    y = np.stack([res[c]["y"].reshape(2, 512) for c in range(NCORES)])
    return np.ascontiguousarray(y.reshape(16, 512).astype(np.float32))
